# revision 1
# baseline (speedup 1.0000x reference)
"""Causal self-attention Trainium2 kernel (Bass/Tile), 8 NeuronCores.

Problem: B=2, S=2048, D=1024, H=16 heads (hd=64), fp32.
    qkv = x @ qkv_w + qkv_b ; per-head causal attention ; y = out @ out_w + out_b

Sharding (hybrid data x tensor parallel):
    8 cores = 2 batch groups x 4 head groups. Core c handles batch c//4 and
    the 4 heads [4*(c%4) .. 4*(c%4)+3]. Each core computes its partial
    out-projection y_c [S, D]; host sums the 4 partials per batch + out_b.

Per-core layout strategy (everything contraction-friendly, zero on-chip
transposes):
    - host supplies xT = x[b].T [D, S] so D is the DMA partition dim
    - qkv^T is computed directly: qkT [hd_n on partitions, S free]
    - scores are computed transposed: sT[k, q] = kT.T @ qT, softmax uses no
      max-subtraction (scores are O(6) so exp is safe in fp32), the softmax
      denominator comes out of the PV matmul via a ones-column appended to V,
      and the normalization divides after PV.
    - out^T accumulates in [hd_local=256 partitions, S] layout, which is
      exactly the lhsT the out-projection needs.
Matmuls run as float32r (full-rate fp32 path on TRN2 PE for free dim >= 256).
"""

import os
import sys

for _p in ("/opt/trn_rl_repo", "/root/.axon_site/_ro/trn_rl_repo"):
    if os.path.isdir(_p) and _p not in sys.path:
        sys.path.insert(0, _p)

import numpy as np
from contextlib import ExitStack

import concourse.bass as bass
import concourse.tile as tile
from concourse import bacc, mybir
from concourse.bass_utils import run_bass_kernel_spmd

B, S, D = 2, 2048, 1024
H, HD = 16, 64
NCORES = 8
LOCAL_H = 4           # heads per core
P = 128
KO = D // P           # 8 contraction sub-tiles for the projections
NQ = S // 512         # 4 q-tiles of 512
NKT = S // P          # 16 k-blocks of 128
F32 = mybir.dt.float32
F32R = mybir.dt.float32r
AF = mybir.ActivationFunctionType
SCALE = 1.0 / np.sqrt(HD)


def _emit(tc, nc, xT, wqk, wv, wo, bqkv, b65, onesd, y, has_qkv_bias):
    with ExitStack() as ctx:
        consts = ctx.enter_context(tc.tile_pool(name="consts", bufs=1))
        persis = ctx.enter_context(tc.tile_pool(name="persist", bufs=1))
        psum = ctx.enter_context(tc.tile_pool(name="ps", bufs=2, space="PSUM"))
        psum_o = ctx.enter_context(tc.tile_pool(name="pso", bufs=2, space="PSUM"))
        xstack = ctx.enter_context(ExitStack())
        xpool = xstack.enter_context(tc.tile_pool(name="xp", bufs=KO))

        # ---- constant loads (wqk/x interleaved per-ko so qkT starts early) ----
        b65_sb = consts.tile([1, 260], F32R)
        nc.scalar.dma_start(b65_sb[:], b65[None, :])
        ones_col = consts.tile([1, P], F32R)
        nc.scalar.dma_start(ones_col[:], onesd[None, :])
        # lower-triangle keep-mask for diagonal 128x128 score blocks
        mask128 = consts.tile([P, P], F32R)
        nc.scalar.dma_start(mask128[:], onesd[None, :].to_broadcast((P, P)))
        nc.gpsimd.affine_select(
            out=mask128[:], in_=mask128[:], pattern=[[1, P]],
            compare_op=mybir.AluOpType.is_ge, fill=0.0, base=0,
            channel_multiplier=-1,
        )
        if has_qkv_bias:
            bqk_sb = consts.tile([P, 4], F32)
            nc.scalar.dma_start(bqk_sb[:], bqkv[0:512].rearrange("(m p) -> p m", p=P))

        x_sb, wqk_t, wv_t = [], [], []
        for ko in range(KO):
            w = consts.tile([P, 512], F32R, name=f"wqk{ko}")
            nc.sync.dma_start(w[:], wqk[ko * P:(ko + 1) * P, :])
            wqk_t.append(w)
            t = xpool.tile([P, S], F32R, tag="x")
            nc.sync.dma_start(t[:], xT[ko * P:(ko + 1) * P, :])
            x_sb.append(t)
        for ko in range(KO):
            w = consts.tile([P, 260], F32R, name=f"wv{ko}")
            nc.sync.dma_start(w[:], wv[ko * P:(ko + 1) * P, :])
            wv_t.append(w)
        wo_sb = consts.tile([P, 2, D], F32R)
        nc.sync.dma_start(wo_sb[:], wo.rearrange("(ks p) n -> p ks n", p=P))

        # persistent activations
        qkT = persis.tile([P, 4, S], F32R)       # m-tiles 0,1: qT(h0..h3); 2,3: kT
        v_all = persis.tile([P, NKT, LOCAL_H, 65], F32R)  # [k-part, kt, lh, hd|ones]
        outT = persis.tile([P, 2, S], F32R)      # attention out^T (out-proj lhsT)

        # ---- qk^T projection: qkT[m] = (wqk[:, m-slice]).T @ xT ----
        for m in range(4):
            for n in range(NQ):
                gidx = m * NQ + n
                pool_ = psum if gidx % 2 == 0 else psum_o
                ps = pool_.tile([P, 512], F32, tag="mm512" if gidx % 2 == 0 else "o",
                                name=f"qk{gidx}")
                for ko in range(KO):
                    nc.tensor.matmul(
                        ps[:],
                        (wqk_t[ko][:, m * P:(m + 1) * P]),
                        (x_sb[ko][:, n * 512:(n + 1) * 512]),
                        start=(ko == 0), stop=(ko == KO - 1),
                    )
                dst = qkT[:, m, n * 512:(n + 1) * 512]
                if has_qkv_bias:
                    nc.scalar.activation(dst, ps[:], AF.Identity, bias=bqk_sb[:, m:m + 1])
                else:
                    nc.vector.tensor_copy(dst, ps[:])

        # ---- v projection (natural layout, ones/bias col via K=1 matmul) ----
        for mt in range(NKT):
            pool_ = psum if mt % 2 == 0 else psum_o
            ps = pool_.tile([P, 512], F32, tag="mm512" if mt % 2 == 0 else "o",
                            name=f"vp{mt}")
            pv = ps[:, 0:260]
            for ko in range(KO):
                nc.tensor.matmul(
                    pv,
                    (x_sb[ko][:, mt * P:(mt + 1) * P]),
                    (wv_t[ko][:]),
                    start=(ko == 0), stop=False,
                )
            nc.tensor.matmul(pv, (ones_col[:1, :]), (b65_sb[:1, :]),
                             start=False, stop=True)
            nc.vector.tensor_copy(
                v_all[:, mt, :, :],
                pv.rearrange("p (h d) -> p h d", h=LOCAL_H),
            )

        # x tiles are dead now; release their SBUF for the attention pools
        xstack.close()
        work = ctx.enter_context(tc.tile_pool(name="work", bufs=4))
        small = ctx.enter_context(tc.tile_pool(name="small", bufs=3))

        # ---- attention (jq outer so out-proj can stream per q-tile) ----
        for jq in range(NQ):
            for hp in range(2):        # local heads (2hp, 2hp+1)
                po = [psum_o.tile([65, 512], F32, tag="o", name=f"po{i_}")
                      for i_ in range(2)]
                last_kt = 4 * jq + 3
                for kt in range(last_kt + 1):
                    # diagonal blocks: columns q < 128*rel are fully masked;
                    # compute only [f0, 512) and mask just the 128-wide triangle
                    rel = kt - 4 * jq
                    f0 = 128 * rel if rel > 0 else 0
                    ps = psum.tile([P, 2, 512], F32, tag="s")
                    for i in range(2):
                        poff = 64 * i
                        nc.tensor.matmul(
                            ps[:, i, f0:512],
                            (qkT[poff:poff + 64, 2 + hp, kt * P:(kt + 1) * P]),
                            (qkT[poff:poff + 64, hp,
                                 jq * 512 + f0:(jq + 1) * 512]),
                            start=True, stop=True,
                        )
                    et = work.tile([P, 2, 512], F32R, tag="e")
                    nc.scalar.activation(et[:, :, f0:512], ps[:, :, f0:512],
                                         AF.Exp, scale=float(SCALE))
                    if rel >= 0:   # mask the 128-wide triangle at [f0, f0+128)
                        nc.vector.tensor_tensor(
                            et[:, 0, f0:f0 + 128], et[:, 0, f0:f0 + 128],
                            mask128[:], mybir.AluOpType.mult)
                        nc.vector.tensor_tensor(
                            et[:, 1, f0:f0 + 128], et[:, 1, f0:f0 + 128],
                            mask128[:], mybir.AluOpType.mult)
                    for i in range(2):
                        lh = 2 * hp + i
                        nc.tensor.matmul(
                            po[i][:, f0:512],
                            (v_all[:, kt, lh, :]),
                            (et[:, i, f0:512]),
                            start=(kt == 0), stop=(kt == last_kt),
                        )
                # stage po out of PSUM immediately (frees the bank for the
                # next head-pair), then normalize off-PSUM.
                # 1/l split across engines: i=0 DVE reciprocal, i=1 ACT
                # exp(-ln(l)) (Ln/Exp share the loaded table set).
                for i in range(2):
                    st = work.tile([65, 512], F32, tag="st")
                    nc.vector.tensor_copy(st[:], po[i][:])
                    rr = small.tile([1, 512], F32R, tag="rr")
                    if i == 0:
                        rf = small.tile([1, 512], F32, tag="rf")
                        nc.vector.reciprocal(rf[:], st[64:65, :])
                        nc.vector.tensor_copy(rr[:], rf[:])
                    else:
                        lr = small.tile([1, 512], F32, tag="lr")
                        nc.scalar.activation(lr[:], st[64:65, :], AF.Ln)
                        nc.scalar.activation(rr[:], lr[:], AF.Exp, scale=-1.0)
                    rb_ps = psum_o.tile([64, 512], F32, tag="o", name="rbps")
                    nc.tensor.matmul(rb_ps[:], ones_col[:1, 0:64], rr[:1, :],
                                     start=True, stop=True)
                    nc.vector.tensor_tensor(
                        outT[64 * i:64 * i + 64, hp, jq * 512:(jq + 1) * 512],
                        st[0:64, :], rb_ps[:], mybir.AluOpType.mult,
                    )
            # ---- out-projection for this q-tile's 4 seq sub-tiles ----
            for mt in range(4 * jq, 4 * jq + 4):
                for n2 in range(2):
                    ps = psum.tile([P, 512], F32, tag="mm512")
                    for ks in range(2):
                        nc.tensor.matmul(
                            ps[:],
                            (outT[:, ks, mt * P:(mt + 1) * P]),
                            (wo_sb[:, ks, n2 * 512:(n2 + 1) * 512]),
                            start=(ks == 0), stop=(ks == 1),
                        )
                    yt = work.tile([P, 512], F32, tag="y")
                    nc.vector.tensor_copy(yt[:], ps[:])
                    nc.gpsimd.dma_start(
                        y[mt * P:(mt + 1) * P, n2 * 512:(n2 + 1) * 512], yt[:])


def build_nc(has_qkv_bias):
    nc = bacc.Bacc("TRN2", target_bir_lowering=False, debug=False,
                   num_devices=NCORES)
    xT = nc.dram_tensor("xT", [D, S], F32R, kind="ExternalInput")
    wqk = nc.dram_tensor("wqk", [D, 512], F32R, kind="ExternalInput")
    wv = nc.dram_tensor("wv", [D, 260], F32R, kind="ExternalInput")
    wo = nc.dram_tensor("wo", [2 * P, D], F32R, kind="ExternalInput")
    bqkv = nc.dram_tensor("bqkv", [768], F32, kind="ExternalInput")
    b65 = nc.dram_tensor("b65", [260], F32R, kind="ExternalInput")
    onesd = nc.dram_tensor("onesd", [P], F32R, kind="ExternalInput")
    y = nc.dram_tensor("y", [S, D], F32, kind="ExternalOutput")
    with tile.TileContext(nc) as tc:
        _emit(tc, nc, xT.ap(), wqk.ap(), wv.ap(), wo.ap(), bqkv.ap(), b65.ap(),
              onesd.ap(), y.ap(), has_qkv_bias)
    nc.compile()
    return nc


_NC_CACHE = {}


def _get_nc(has_qkv_bias):
    key = bool(has_qkv_bias)
    if key not in _NC_CACHE:
        _NC_CACHE[key] = build_nc(key)
    return _NC_CACHE[key]


def _round_fp32r(a):
    """Round fp32 to the fp32r grid (11-bit mantissa; low 12 bits zero, RNE)."""
    u = np.ascontiguousarray(a, dtype=np.float32).view(np.uint32)
    u = (u + 0x7FF + ((u >> 12) & 1)) & np.uint32(0xFFFFF000)
    return u.view(np.float32)


def make_in_maps(x, qkv_w, qkv_b, out_w):
    """Per-core host-side sharding. Core c: batch c//4, heads 4*(c%4)..+3."""
    in_maps = []
    xTs = [_round_fp32r(np.ascontiguousarray(x[b].T)) for b in range(B)]
    for c in range(NCORES):
        b = c // (NCORES // B)
        g = c % (NCORES // B)
        h0 = LOCAL_H * g
        cols = slice(h0 * HD, (h0 + LOCAL_H) * HD)
        wq = qkv_w[:, cols]
        wk = qkv_w[:, D:][:, cols]
        wv_ = qkv_w[:, 2 * D:][:, cols]
        bq = qkv_b[cols]
        bk = qkv_b[D:][cols]
        bv = qkv_b[2 * D:][cols]
        wv_pad = np.zeros((D, LOCAL_H, 65), np.float32)
        wv_pad[:, :, :64] = wv_.reshape(D, LOCAL_H, HD)
        b65_arr = np.zeros((LOCAL_H, 65), np.float32)
        b65_arr[:, :64] = bv.reshape(LOCAL_H, HD)
        b65_arr[:, 64] = 1.0
        in_maps.append({
            "xT": xTs[b],
            "wqk": _round_fp32r(np.concatenate([wq, wk], axis=1)),
            "wv": _round_fp32r(wv_pad.reshape(D, LOCAL_H * 65)),
            "wo": _round_fp32r(out_w[cols, :]),
            "bqkv": np.ascontiguousarray(np.concatenate([bq, bk, bv])),
            "b65": _round_fp32r(b65_arr.reshape(-1)),
            "onesd": np.ones(P, np.float32),
        })
    return in_maps


def _ensure_ntff_hook():
    """Provide antenv.axon_hooks (missing in this image) so trace=True works."""
    try:
        from antenv.axon_hooks import get_axon_ntff_profile_hook  # noqa: F401
        return
    except ImportError:
        pass
    import types
    import antenv
    mod = types.ModuleType("antenv.axon_hooks")
    holder = {"hook": None}
    mod.set_axon_ntff_profile_hook = lambda h: holder.__setitem__("hook", h)
    mod.get_axon_ntff_profile_hook = lambda: holder["hook"]
    sys.modules["antenv.axon_hooks"] = mod
    antenv.axon_hooks = mod
    try:
        from trn_agent_boot.trn_boot import _ntff_profile_via_ctypes
        so = "/opt/axon/libaxon_pjrt.so"
        if os.path.exists(so):
            mod.set_axon_ntff_profile_hook(_ntff_profile_via_ctypes(so))
    except Exception:
        pass


def kernel(x, qkv_w, qkv_b, out_w, out_b, _trace=False):
    if _trace:
        _ensure_ntff_hook()
    x = np.asarray(x, dtype=np.float32)
    qkv_w = np.asarray(qkv_w, dtype=np.float32)
    qkv_b = np.asarray(qkv_b, dtype=np.float32)
    out_w = np.asarray(out_w, dtype=np.float32)
    out_b = np.asarray(out_b, dtype=np.float32)

    has_qkv_bias = bool(np.any(qkv_b))
    nc = _get_nc(has_qkv_bias)
    in_maps = make_in_maps(x, qkv_w, qkv_b, out_w)
    res = run_bass_kernel_spmd(nc, in_maps, core_ids=list(range(NCORES)),
                               trace=_trace)
    y = np.zeros((B, S, D), dtype=np.float32)
    for c in range(NCORES):
        y[c // (NCORES // B)] += res.results[c]["y"]
    y += out_b
    if _trace:
        kernel.last_results = res
    return y



# revision 7
# speedup vs baseline: 1.2383x; 1.2383x over previous
"""Causal self-attention Trainium2 kernel (Bass/Tile), 8 NeuronCores.

Problem: B=2, S=2048, D=1024, H=16 heads (hd=64), fp32.
    qkv = x @ qkv_w + qkv_b ; per-head causal attention ; y = out @ out_proj + out_b

Sharding (hybrid data x tensor parallel):
    8 cores = 2 batch groups x 4 head groups. Core c handles batch c//4 and
    the 4 heads [4*(c%4) .. 4*(c%4)+3]. Each core computes its partial
    out-projection y_c [S, D] in bf16; host sums the 4 partials per batch
    (in fp32) + out_b.

v2 design (bf16 everywhere on the PE, balanced engine usage):
    - all matmuls in bf16 (full PE rate at any free size, half the DMA/SBUF)
    - phase 1 (projections) streams x per-ko with ko-OUTER accumulation over
      8 PSUM banks so the PE starts as soon as x[0] lands; PSUM->SBUF drains
      run on the otherwise-idle ACT engine
    - softmax: exp on ACT only (no Ln -> no activation-table thrash);
      denominators via the ones-column of V; 1/l via DVE
      reciprocal_approx_fast (~5x faster than nc.vector.reciprocal);
      broadcast of 1/l across partitions via a tiny K=2 f32r matmul
    - causal masking of diagonal 128-blocks via gpsimd.affine_select on the
      exp'd tile (keeps DVE free)
    - softmax-normalize + out-projection of q-tile jq are emitted as deferred
      units interleaved into the next tile's attention loop, so the PE never
      stalls on the normalization chain
"""

import os
import sys
from collections import deque

for _p in ("/opt/trn_rl_repo", "/root/.axon_site/_ro/trn_rl_repo"):
    if os.path.isdir(_p) and _p not in sys.path:
        sys.path.insert(0, _p)

import numpy as np
import ml_dtypes
from contextlib import ExitStack

import concourse.bass as bass
import concourse.tile as tile
from concourse import bacc, mybir
from concourse.bass_utils import run_bass_kernel_spmd

B, S, D = 2, 2048, 1024
H, HD = 16, 64
NCORES = 8
LOCAL_H = 4           # heads per core
P = 128
KO = D // P           # 8 contraction sub-tiles for the projections
NQ = S // 512         # 4 q-tiles of 512
NKT = S // P          # 16 k-blocks of 128
F32 = mybir.dt.float32
F32R = mybir.dt.float32r
BF16 = mybir.dt.bfloat16
AF = mybir.ActivationFunctionType
ALU = mybir.AluOpType
SCALE = 1.0 / np.sqrt(HD)


def _emit(tc, nc, xT, wqk, wv, wo, bqk, b65, onesd, sel2, y, has_qkv_bias):
    with ExitStack() as ctx:
        consts = ctx.enter_context(tc.tile_pool(name="consts", bufs=1))
        persis = ctx.enter_context(tc.tile_pool(name="persist", bufs=1))
        xstack = ctx.enter_context(ExitStack())
        xpool = xstack.enter_context(tc.tile_pool(name="xp", bufs=KO))
        ppstack = ctx.enter_context(ExitStack())
        pp = ppstack.enter_context(tc.tile_pool(name="pp", bufs=8, space="PSUM"))

        # ---- constant + weight loads (ACT sequencer) ----
        wqk_sb = consts.tile([P, KO, 512], BF16)
        nc.scalar.dma_start(wqk_sb[:], wqk.rearrange("(ko p) m -> p ko m", p=P))
        b65_sb = consts.tile([1, 260], BF16)
        nc.scalar.dma_start(b65_sb[:], b65[None, :])
        ones_sb = consts.tile([1, P], BF16)
        nc.scalar.dma_start(ones_sb[:], onesd[None, :])
        onesr_sb = consts.tile([1, P], F32R)
        nc.scalar.dma_start(onesr_sb[:], sel2[0:1, :])
        wv_sb = consts.tile([P, KO, 260], BF16)
        nc.scalar.dma_start(wv_sb[:], wv.rearrange("(ko p) m -> p ko m", p=P))
        wo_sb = consts.tile([P, 2, D], BF16)
        nc.scalar.dma_start(wo_sb[:], wo.rearrange("(ks p) n -> p ks n", p=P))
        if has_qkv_bias:
            bqk_sb = consts.tile([P, 4], F32)
            nc.scalar.dma_start(bqk_sb[:], bqk.rearrange("(m p) -> p m", p=P))

        # ---- x tiles, streamed per-ko (SP sequencer) ----
        x_sb = []
        for ko in range(KO):
            t = xpool.tile([P, S], BF16, tag="x", name=f"x{ko}")
            nc.sync.dma_start(t[:], xT[ko * P:(ko + 1) * P, :])
            x_sb.append(t)

        # persistent activations
        qkT = persis.tile([P, 4, S], BF16)       # m 0,1: qT(h0..h3); 2,3: kT
        v_all = persis.tile([P, NKT, LOCAL_H, 65], BF16)  # [k-part, kt, lh, hd|ones]
        outT = persis.tile([P, 2, S], BF16)      # attention out^T (out-proj lhsT)

        # ---- phase 1: projections, ko-outer over 8 PSUM banks ----
        # qkT[m] = (wqk[:, m-slice]).T @ xT, two passes of 8 (m, n) groups
        for half in range(2):
            groups = [(m, 2 * half + nn) for m in range(4) for nn in range(2)]
            ts = [pp.tile([P, 512], F32, tag="p", name=f"qk{half}_{g}")
                  for g in range(8)]
            for ko in range(KO):
                for g, (m, n) in enumerate(groups):
                    nc.tensor.matmul(
                        ts[g][:],
                        wqk_sb[:, ko, m * P:(m + 1) * P],
                        x_sb[ko][:, n * 512:(n + 1) * 512],
                        start=(ko == 0), stop=(ko == KO - 1),
                    )
            for g, (m, n) in enumerate(groups):
                dst = qkT[:, m, n * 512:(n + 1) * 512]
                if has_qkv_bias:
                    nc.scalar.activation(dst, ts[g][:], AF.Identity,
                                         bias=bqk_sb[:, m:m + 1])
                else:
                    nc.scalar.copy(dst, ts[g][:])

        # v projection (natural layout, ones/bias row via K=1 matmul)
        for half in range(2):
            mts = [8 * half + g for g in range(8)]
            ts = [pp.tile([P, 512], F32, tag="p", name=f"v{half}_{g}")
                  for g in range(8)]
            for ko in range(KO):
                for g, mt in enumerate(mts):
                    nc.tensor.matmul(
                        ts[g][:, 0:260],
                        x_sb[ko][:, mt * P:(mt + 1) * P],
                        wv_sb[:, ko, :],
                        start=(ko == 0), stop=False,
                    )
            for g, mt in enumerate(mts):
                nc.tensor.matmul(ts[g][:, 0:260], ones_sb[:1, :], b65_sb[:1, :],
                                 start=False, stop=True)
                nc.scalar.copy(
                    v_all[:, mt, :, :],
                    ts[g][:, 0:260].rearrange("p (h d) -> p h d", h=LOCAL_H),
                )

        # x tiles + phase-1 psum are dead; release for the attention pools
        xstack.close()
        ppstack.close()

        psA = ctx.enter_context(tc.tile_pool(name="psA", bufs=2, space="PSUM"))
        psB = ctx.enter_context(tc.tile_pool(name="psB", bufs=4, space="PSUM"))
        work = ctx.enter_context(tc.tile_pool(name="work", bufs=4))
        small = ctx.enter_context(tc.tile_pool(name="small", bufs=2))
        ypool = ctx.enter_context(tc.tile_pool(name="yp", bufs=4))

        units = deque()

        def unit_rb_norm(jq, hp, po, rrr, i):
            def emit():
                rbp = psA.tile([P, 2, 512], F32, tag="s", name="rbp")
                nc.tensor.matmul(rbp[0:64, 0, :], onesr_sb[:1, 0:64],
                                 rrr[:1, :], start=True, stop=True)
                rb = small.tile([64, 512], F32, tag="rb", name="rb")
                nc.vector.tensor_copy(rb[:], rbp[0:64, 0, :])
                nc.vector.tensor_tensor(
                    outT[64 * i:64 * i + 64, hp, jq * 512:(jq + 1) * 512],
                    po[i][0:64, :], rb[:], ALU.mult)
            return emit

        def unit_outproj(jq, mt, n2):
            def emit():
                pso = psA.tile([P, 2, 512], F32, tag="s", name="pso")[:, 0, :]
                for ks in range(2):
                    nc.tensor.matmul(
                        pso,
                        outT[:, ks, mt * P:(mt + 1) * P],
                        wo_sb[:, ks, n2 * 512:(n2 + 1) * 512],
                        start=(ks == 0), stop=(ks == 1),
                    )
                yt = ypool.tile([P, 512], BF16, tag="y", name="yt")
                nc.vector.tensor_copy(yt[:], pso)
                nc.sync.dma_start(
                    y[mt * P:(mt + 1) * P, n2 * 512:(n2 + 1) * 512], yt[:])
            return emit

        def emit_pv(po, jq, hp, kt, et, f0, last_kt):
            for i in range(2):
                nc.tensor.matmul(
                    po[i][:, f0:512],
                    v_all[:, kt, 2 * hp + i, :],
                    et[:, i, f0:512],
                    start=(kt == 0), stop=(kt == last_kt),
                )

        # ---- phase 2: attention with deferred normalize/out-proj units ----
        for jq in range(NQ):
            for hp in range(2):
                last_kt = 4 * jq + 3
                po = [psB.tile([65, 512], F32, tag="o", name=f"po{jq}{hp}{i_}")
                      for i_ in range(2)]
                prev = None
                for kt in range(last_kt + 1):
                    rel = kt - 4 * jq
                    f0 = 128 * rel if rel > 0 else 0
                    ps = psA.tile([P, 2, 512], F32, tag="s", name="ps")
                    for i in range(2):
                        poff = 64 * i
                        nc.tensor.matmul(
                            ps[:, i, f0:512],
                            qkT[poff:poff + 64, 2 + hp, kt * P:(kt + 1) * P],
                            qkT[poff:poff + 64, hp,
                                jq * 512 + f0:(jq + 1) * 512],
                            start=True, stop=True,
                        )
                    et = work.tile([P, 2, 512], BF16, tag="e", name="et")
                    nc.scalar.activation(et[:, :, f0:512], ps[:, :, f0:512],
                                         AF.Exp, scale=float(SCALE))
                    if rel >= 0:   # zero the 128-wide triangle at [f0, f0+128)
                        nc.gpsimd.affine_select(
                            out=et[:, :, f0:f0 + 128],
                            in_=et[:, :, f0:f0 + 128],
                            pattern=[[0, 2], [1, P]],
                            compare_op=ALU.is_ge, fill=0.0, base=0,
                            channel_multiplier=-1,
                        )
                    if prev is not None:
                        emit_pv(po, jq, hp, *prev, last_kt)
                        if units:
                            units.popleft()()
                    prev = (kt, et, f0)
                emit_pv(po, jq, hp, *prev, last_kt)
                # denominators: l rows -> 1/l (fast approx), cast to f32r
                for i in range(2):
                    lcp = small.tile([1, 512], F32, tag="lcp", name="lcp")
                    nc.vector.tensor_copy(lcp[:], po[i][64:65, :])
                    rr = small.tile([1, 512], F32, tag="rr", name="rr")
                    nc.vector.reciprocal_approx_fast(rr[:], lcp[:])
                    rrr = small.tile([1, 512], F32R, tag="rrr", name="rrr")
                    nc.vector.tensor_copy(rrr[:], rr[:])
                    units.append(unit_rb_norm(jq, hp, po, rrr, i))
            for mt in range(4 * jq, 4 * jq + 4):
                for n2 in range(2):
                    units.append(unit_outproj(jq, mt, n2))
        while units:
            units.popleft()()


def build_nc(has_qkv_bias):
    nc = bacc.Bacc("TRN2", target_bir_lowering=False, debug=False,
                   num_devices=NCORES)
    xT = nc.dram_tensor("xT", [D, S], BF16, kind="ExternalInput")
    wqk = nc.dram_tensor("wqk", [D, 512], BF16, kind="ExternalInput")
    wv = nc.dram_tensor("wv", [D, 260], BF16, kind="ExternalInput")
    wo = nc.dram_tensor("wo", [2 * P, D], BF16, kind="ExternalInput")
    bqk = nc.dram_tensor("bqk", [512], F32, kind="ExternalInput")
    b65 = nc.dram_tensor("b65", [260], BF16, kind="ExternalInput")
    onesd = nc.dram_tensor("onesd", [P], BF16, kind="ExternalInput")
    sel2 = nc.dram_tensor("sel2", [2, P], F32R, kind="ExternalInput")
    y = nc.dram_tensor("y", [S, D], BF16, kind="ExternalOutput")
    with tile.TileContext(nc) as tc:
        _emit(tc, nc, xT.ap(), wqk.ap(), wv.ap(), wo.ap(), bqk.ap(), b65.ap(),
              onesd.ap(), sel2.ap(), y.ap(), has_qkv_bias)
    nc.compile()
    return nc


_NC_CACHE = {}


def _get_nc(has_qkv_bias):
    key = bool(has_qkv_bias)
    if key not in _NC_CACHE:
        _NC_CACHE[key] = build_nc(key)
    return _NC_CACHE[key]


def _bf16(a):
    return np.ascontiguousarray(a, dtype=np.float32).astype(ml_dtypes.bfloat16)


def make_in_maps(x, qkv_w, qkv_b, out_w):
    """Per-core host-side sharding. Core c: batch c//4, heads 4*(c%4)..+3."""
    in_maps = []
    xTs = [_bf16(x[b].T) for b in range(B)]
    sel2 = np.zeros((2, P), np.float32)
    sel2[0, 0:64] = 1.0
    sel2[1, 64:128] = 1.0
    for c in range(NCORES):
        b = c // (NCORES // B)
        g = c % (NCORES // B)
        h0 = LOCAL_H * g
        cols = slice(h0 * HD, (h0 + LOCAL_H) * HD)
        wq = qkv_w[:, cols]
        wk = qkv_w[:, D:][:, cols]
        wv_ = qkv_w[:, 2 * D:][:, cols]
        bq = qkv_b[cols]
        bk = qkv_b[D:][cols]
        bv = qkv_b[2 * D:][cols]
        wv_pad = np.zeros((D, LOCAL_H, 65), np.float32)
        wv_pad[:, :, :64] = wv_.reshape(D, LOCAL_H, HD)
        b65_arr = np.zeros((LOCAL_H, 65), np.float32)
        b65_arr[:, :64] = bv.reshape(LOCAL_H, HD)
        b65_arr[:, 64] = 1.0
        in_maps.append({
            "xT": xTs[b],
            "wqk": _bf16(np.concatenate([wq, wk], axis=1)),
            "wv": _bf16(wv_pad.reshape(D, LOCAL_H * 65)),
            "wo": _bf16(out_w[cols, :]),
            "bqk": np.ascontiguousarray(np.concatenate([bq, bk])),
            "b65": _bf16(b65_arr.reshape(-1)),
            "onesd": np.ones(P, ml_dtypes.bfloat16),
            "sel2": sel2,
        })
    return in_maps


def _ensure_ntff_hook():
    """Provide antenv.axon_hooks (missing in this image) so trace=True works."""
    try:
        from antenv.axon_hooks import get_axon_ntff_profile_hook  # noqa: F401
        return
    except ImportError:
        pass
    import types
    import antenv
    mod = types.ModuleType("antenv.axon_hooks")
    holder = {"hook": None}
    mod.set_axon_ntff_profile_hook = lambda h: holder.__setitem__("hook", h)
    mod.get_axon_ntff_profile_hook = lambda: holder["hook"]
    sys.modules["antenv.axon_hooks"] = mod
    antenv.axon_hooks = mod
    try:
        from trn_agent_boot.trn_boot import _ntff_profile_via_ctypes
        so = "/opt/axon/libaxon_pjrt.so"
        if os.path.exists(so):
            mod.set_axon_ntff_profile_hook(_ntff_profile_via_ctypes(so))
    except Exception:
        pass


def kernel(x, qkv_w, qkv_b, out_w, out_b, _trace=False):
    if _trace:
        _ensure_ntff_hook()
    x = np.asarray(x, dtype=np.float32)
    qkv_w = np.asarray(qkv_w, dtype=np.float32)
    qkv_b = np.asarray(qkv_b, dtype=np.float32)
    out_w = np.asarray(out_w, dtype=np.float32)
    out_b = np.asarray(out_b, dtype=np.float32)

    has_qkv_bias = bool(np.any(qkv_b[:2 * D]))
    nc = _get_nc(has_qkv_bias)
    in_maps = make_in_maps(x, qkv_w, qkv_b, out_w)
    res = run_bass_kernel_spmd(nc, in_maps, core_ids=list(range(NCORES)),
                               trace=_trace)
    y = np.zeros((B, S, D), dtype=np.float32)
    for c in range(NCORES):
        y[c // (NCORES // B)] += np.asarray(res.results[c]["y"],
                                            dtype=np.float32)
    y += out_b
    if _trace:
        kernel.last_results = res
    return y


# revision 11
# speedup vs baseline: 1.3186x; 1.0649x over previous
"""Causal self-attention Trainium2 kernel (Bass/Tile), 8 NeuronCores.

Problem: B=2, S=2048, D=1024, H=16 heads (hd=64), fp32.
    qkv = x @ qkv_w + qkv_b ; per-head causal attention ; y = out @ out_proj + out_b

Sharding (hybrid data x tensor parallel):
    8 cores = 2 batch groups x 4 head groups. Core c handles batch c//4 and
    the 4 heads [4*(c%4) .. 4*(c%4)+3]. Each core computes its partial
    out-projection y_c [S, D] in bf16; host sums the 4 partials per batch
    (in fp32) + out_b.

v2 design (bf16 everywhere on the PE, balanced engine usage):
    - all matmuls in bf16 (full PE rate at any free size, half the DMA/SBUF)
    - phase 1 (projections) streams x per-ko with ko-OUTER accumulation over
      8 PSUM banks so the PE starts as soon as x[0] lands; PSUM->SBUF drains
      run on the otherwise-idle ACT engine
    - softmax: exp on ACT only (no Ln -> no activation-table thrash);
      denominators via the ones-column of V; 1/l via DVE
      reciprocal_approx_fast (~5x faster than nc.vector.reciprocal);
      broadcast of 1/l across partitions via a tiny K=2 f32r matmul
    - causal masking of diagonal 128-blocks via gpsimd.affine_select on the
      exp'd tile (keeps DVE free)
    - softmax-normalize + out-projection of q-tile jq are emitted as deferred
      units interleaved into the next tile's attention loop, so the PE never
      stalls on the normalization chain
"""

import os
import sys
from collections import deque

for _p in ("/opt/trn_rl_repo", "/root/.axon_site/_ro/trn_rl_repo"):
    if os.path.isdir(_p) and _p not in sys.path:
        sys.path.insert(0, _p)

import numpy as np
import ml_dtypes
from contextlib import ExitStack

import concourse.bass as bass
import concourse.tile as tile
from concourse import bacc, mybir
from concourse.bass_utils import run_bass_kernel_spmd

B, S, D = 2, 2048, 1024
H, HD = 16, 64
NCORES = 8
LOCAL_H = 4           # heads per core
P = 128
KO = D // P           # 8 contraction sub-tiles for the projections
NQ = S // 512         # 4 q-tiles of 512
NKT = S // P          # 16 k-blocks of 128
F32 = mybir.dt.float32
F32R = mybir.dt.float32r
BF16 = mybir.dt.bfloat16
AF = mybir.ActivationFunctionType
ALU = mybir.AluOpType
SCALE = 1.0 / np.sqrt(HD)


def _emit(tc, nc, xT, wqk, wv, wo, bqk, b65, onesd, sel2, y, has_qkv_bias):
    with ExitStack() as ctx:
        consts = ctx.enter_context(tc.tile_pool(name="consts", bufs=1))
        persis = ctx.enter_context(tc.tile_pool(name="persist", bufs=1))
        xstack = ctx.enter_context(ExitStack())
        xpool = xstack.enter_context(tc.tile_pool(name="xp", bufs=KO))
        ppstack = ctx.enter_context(ExitStack())
        pp = ppstack.enter_context(tc.tile_pool(name="pp", bufs=8, space="PSUM"))

        # ---- loads: wqk + early x tiles first (weights for later phases
        # are issued after the x stream so they don't steal DMA bandwidth)
        wqk_sb = consts.tile([P, KO, 512], BF16)
        nc.gpsimd.dma_start(wqk_sb[:], wqk.rearrange("(ko p) m -> p ko m", p=P))
        x_sb = []
        for ko in range(KO):
            t = xpool.tile([P, S], BF16, tag="x", name=f"x{ko}")
            x_sb.append(t)
        for ko in range(4):
            nc.sync.dma_start(x_sb[ko][:], xT[ko * P:(ko + 1) * P, :])
        b65_sb = consts.tile([1, 260], BF16)
        nc.scalar.dma_start(b65_sb[:], b65[None, :])
        ones_sb = consts.tile([1, P], BF16)
        nc.scalar.dma_start(ones_sb[:], onesd[None, :])
        onesr_sb = consts.tile([1, P], F32R)
        nc.scalar.dma_start(onesr_sb[:], sel2[0:1, :])
        for ko in range(4, KO):
            nc.sync.dma_start(x_sb[ko][:], xT[ko * P:(ko + 1) * P, :])
        wv_sb = consts.tile([P, KO, 260], BF16)
        nc.scalar.dma_start(wv_sb[:], wv.rearrange("(ko p) m -> p ko m", p=P))
        wo_sb = consts.tile([P, 2, D], BF16)
        nc.scalar.dma_start(wo_sb[:], wo.rearrange("(ks p) n -> p ks n", p=P))
        if has_qkv_bias:
            bqk_sb = consts.tile([P, 4], F32)
            nc.scalar.dma_start(bqk_sb[:], bqk.rearrange("(m p) -> p m", p=P))

        # persistent activations
        qkT = persis.tile([P, 4, S], BF16)       # m 0,1: qT(h0..h3); 2,3: kT
        v_all = persis.tile([P, NKT, LOCAL_H, 65], BF16)  # [k-part, kt, lh, hd|ones]
        outT = persis.tile([P, 2, S], BF16)      # attention out^T (out-proj lhsT)

        # ---- phase 1: projections, ko-outer over 8 PSUM banks ----
        # qkT[m] = (wqk[:, m-slice]).T @ xT, two passes of 8 (m, n) groups
        for half in range(2):
            groups = [(m, 2 * half + nn) for m in range(4) for nn in range(2)]
            ts = [pp.tile([P, 512], F32, tag="p", name=f"qk{half}_{g}")
                  for g in range(8)]
            for ko in range(KO):
                for g, (m, n) in enumerate(groups):
                    nc.tensor.matmul(
                        ts[g][:],
                        wqk_sb[:, ko, m * P:(m + 1) * P],
                        x_sb[ko][:, n * 512:(n + 1) * 512],
                        start=(ko == 0), stop=(ko == KO - 1),
                    )
            for g, (m, n) in enumerate(groups):
                dst = qkT[:, m, n * 512:(n + 1) * 512]
                if has_qkv_bias:
                    nc.scalar.activation(dst, ts[g][:], AF.Identity,
                                         bias=bqk_sb[:, m:m + 1])
                else:
                    nc.scalar.copy(dst, ts[g][:])

        # v projection (natural layout, ones/bias row via K=1 matmul)
        for half in range(2):
            mts = [8 * half + g for g in range(8)]
            ts = [pp.tile([P, 512], F32, tag="p", name=f"v{half}_{g}")
                  for g in range(8)]
            for ko in range(KO):
                for g, mt in enumerate(mts):
                    nc.tensor.matmul(
                        ts[g][:, 0:260],
                        x_sb[ko][:, mt * P:(mt + 1) * P],
                        wv_sb[:, ko, :],
                        start=(ko == 0), stop=False,
                    )
            for g, mt in enumerate(mts):
                nc.tensor.matmul(ts[g][:, 0:260], ones_sb[:1, :], b65_sb[:1, :],
                                 start=False, stop=True)
                nc.scalar.copy(
                    v_all[:, mt, :, :],
                    ts[g][:, 0:260].rearrange("p (h d) -> p h d", h=LOCAL_H),
                )

        # x tiles + phase-1 psum are dead; release for the attention pools
        xstack.close()
        ppstack.close()

        psA = ctx.enter_context(tc.tile_pool(name="psA", bufs=3, space="PSUM"))
        psB = ctx.enter_context(tc.tile_pool(name="psB", bufs=2, space="PSUM"))
        work = ctx.enter_context(tc.tile_pool(name="work", bufs=4))
        small = ctx.enter_context(tc.tile_pool(name="small", bufs=2))
        ypool = ctx.enter_context(tc.tile_pool(name="yp", bufs=4))

        units = deque()

        def unit_rb_norm(jq, hp, st, rrr, i):
            def emit():
                rbp = psA.tile([P, 2, 512], F32, tag="s", name="rbp")
                nc.tensor.matmul(rbp[0:64, 0, :], onesr_sb[:1, 0:64],
                                 rrr[:1, :], start=True, stop=True)
                nc.vector.tensor_tensor(
                    outT[64 * i:64 * i + 64, hp, jq * 512:(jq + 1) * 512],
                    st[0:64, :], rbp[0:64, 0, :], ALU.mult)
            return emit

        def unit_outproj(jq, mt, n2):
            def emit():
                pso = psA.tile([P, 2, 512], F32, tag="s", name="pso")[:, 0, :]
                for ks in range(2):
                    nc.tensor.matmul(
                        pso,
                        outT[:, ks, mt * P:(mt + 1) * P],
                        wo_sb[:, ks, n2 * 512:(n2 + 1) * 512],
                        start=(ks == 0), stop=(ks == 1),
                    )
                yt = ypool.tile([P, 512], BF16, tag="y", name="yt")
                nc.vector.tensor_copy(yt[:], pso)
                nc.sync.dma_start(
                    y[mt * P:(mt + 1) * P, n2 * 512:(n2 + 1) * 512], yt[:])
            return emit

        def emit_pv(po, jq, hp, kt, et, f0, last_kt):
            for i in range(2):
                nc.tensor.matmul(
                    po[i][:, f0:512],
                    v_all[:, kt, 2 * hp + i, :],
                    et[:, i, f0:512],
                    start=(kt == 0), stop=(kt == last_kt),
                )

        # ---- phase 2: attention with deferred normalize/out-proj units ----
        for jq in range(NQ):
            for hp in range(2):
                last_kt = 4 * jq + 3
                po = [psB.tile([65, 512], F32, tag="o", name=f"po{jq}{hp}{i_}")
                      for i_ in range(2)]
                pend = deque()
                for kt in range(last_kt + 1):
                    rel = kt - 4 * jq
                    f0 = 128 * rel if rel > 0 else 0
                    ps = psA.tile([P, 2, 512], F32, tag="s", name="ps")
                    for i in range(2):
                        poff = 64 * i
                        nc.tensor.matmul(
                            ps[:, i, f0:512],
                            qkT[poff:poff + 64, 2 + hp, kt * P:(kt + 1) * P],
                            qkT[poff:poff + 64, hp,
                                jq * 512 + f0:(jq + 1) * 512],
                            start=True, stop=True,
                        )
                    et = work.tile([P, 2, 512], BF16, tag="e", name="et")
                    nc.scalar.activation(et[:, :, f0:512], ps[:, :, f0:512],
                                         AF.Exp, scale=float(SCALE))
                    if rel >= 0:   # zero the 128-wide triangle at [f0, f0+128)
                        nc.gpsimd.affine_select(
                            out=et[:, :, f0:f0 + 128],
                            in_=et[:, :, f0:f0 + 128],
                            pattern=[[0, 2], [1, P]],
                            compare_op=ALU.is_ge, fill=0.0, base=0,
                            channel_multiplier=-1,
                        )
                    pend.append((kt, et, f0))
                    if len(pend) > 2:   # 2-deep score lookahead ahead of PV
                        emit_pv(po, jq, hp, *pend.popleft(), last_kt)
                        if units:
                            units.popleft()()
                while pend:
                    emit_pv(po, jq, hp, *pend.popleft(), last_kt)
                # stage po out of PSUM on the ACT engine (frees the bank
                # fast), grab denominators, 1/l via fast DVE approx
                for i in range(2):
                    stt = work.tile([65, 512], F32, tag="st", name="st")
                    nc.scalar.copy(stt[:], po[i][:])
                    lcp = small.tile([1, 512], F32, tag="lcp", name="lcp")
                    nc.vector.tensor_copy(lcp[:], po[i][64:65, :])
                    rr = small.tile([1, 512], F32, tag="rr", name="rr")
                    nc.vector.reciprocal_approx_fast(rr[:], lcp[:])
                    rrr = small.tile([1, 512], F32R, tag="rrr", name="rrr")
                    nc.vector.tensor_copy(rrr[:], rr[:])
                    units.append(unit_rb_norm(jq, hp, stt, rrr, i))
            for mt in range(4 * jq, 4 * jq + 4):
                for n2 in range(2):
                    units.append(unit_outproj(jq, mt, n2))
        while units:
            units.popleft()()


def build_nc(has_qkv_bias):
    nc = bacc.Bacc("TRN2", target_bir_lowering=False, debug=False,
                   num_devices=NCORES)
    xT = nc.dram_tensor("xT", [D, S], BF16, kind="ExternalInput")
    wqk = nc.dram_tensor("wqk", [D, 512], BF16, kind="ExternalInput")
    wv = nc.dram_tensor("wv", [D, 260], BF16, kind="ExternalInput")
    wo = nc.dram_tensor("wo", [2 * P, D], BF16, kind="ExternalInput")
    bqk = nc.dram_tensor("bqk", [512], F32, kind="ExternalInput")
    b65 = nc.dram_tensor("b65", [260], BF16, kind="ExternalInput")
    onesd = nc.dram_tensor("onesd", [P], BF16, kind="ExternalInput")
    sel2 = nc.dram_tensor("sel2", [2, P], F32R, kind="ExternalInput")
    y = nc.dram_tensor("y", [S, D], BF16, kind="ExternalOutput")
    with tile.TileContext(nc) as tc:
        _emit(tc, nc, xT.ap(), wqk.ap(), wv.ap(), wo.ap(), bqk.ap(), b65.ap(),
              onesd.ap(), sel2.ap(), y.ap(), has_qkv_bias)
    nc.compile()
    return nc


_NC_CACHE = {}


def _get_nc(has_qkv_bias):
    key = bool(has_qkv_bias)
    if key not in _NC_CACHE:
        _NC_CACHE[key] = build_nc(key)
    return _NC_CACHE[key]


def _bf16(a):
    return np.ascontiguousarray(a, dtype=np.float32).astype(ml_dtypes.bfloat16)


def make_in_maps(x, qkv_w, qkv_b, out_w):
    """Per-core host-side sharding. Core c: batch c//4, heads 4*(c%4)..+3."""
    in_maps = []
    xTs = [_bf16(x[b].T) for b in range(B)]
    sel2 = np.zeros((2, P), np.float32)
    sel2[0, 0:64] = 1.0
    sel2[1, 64:128] = 1.0
    for c in range(NCORES):
        b = c // (NCORES // B)
        g = c % (NCORES // B)
        h0 = LOCAL_H * g
        cols = slice(h0 * HD, (h0 + LOCAL_H) * HD)
        wq = qkv_w[:, cols]
        wk = qkv_w[:, D:][:, cols]
        wv_ = qkv_w[:, 2 * D:][:, cols]
        bq = qkv_b[cols]
        bk = qkv_b[D:][cols]
        bv = qkv_b[2 * D:][cols]
        wv_pad = np.zeros((D, LOCAL_H, 65), np.float32)
        wv_pad[:, :, :64] = wv_.reshape(D, LOCAL_H, HD)
        b65_arr = np.zeros((LOCAL_H, 65), np.float32)
        b65_arr[:, :64] = bv.reshape(LOCAL_H, HD)
        b65_arr[:, 64] = 1.0
        in_maps.append({
            "xT": xTs[b],
            "wqk": _bf16(np.concatenate([wq, wk], axis=1)),
            "wv": _bf16(wv_pad.reshape(D, LOCAL_H * 65)),
            "wo": _bf16(out_w[cols, :]),
            "bqk": np.ascontiguousarray(np.concatenate([bq, bk])),
            "b65": _bf16(b65_arr.reshape(-1)),
            "onesd": np.ones(P, ml_dtypes.bfloat16),
            "sel2": sel2,
        })
    return in_maps


def _ensure_ntff_hook():
    """Provide antenv.axon_hooks (missing in this image) so trace=True works."""
    try:
        from antenv.axon_hooks import get_axon_ntff_profile_hook  # noqa: F401
        return
    except ImportError:
        pass
    import types
    import antenv
    mod = types.ModuleType("antenv.axon_hooks")
    holder = {"hook": None}
    mod.set_axon_ntff_profile_hook = lambda h: holder.__setitem__("hook", h)
    mod.get_axon_ntff_profile_hook = lambda: holder["hook"]
    sys.modules["antenv.axon_hooks"] = mod
    antenv.axon_hooks = mod
    try:
        from trn_agent_boot.trn_boot import _ntff_profile_via_ctypes
        so = "/opt/axon/libaxon_pjrt.so"
        if os.path.exists(so):
            mod.set_axon_ntff_profile_hook(_ntff_profile_via_ctypes(so))
    except Exception:
        pass


def kernel(x, qkv_w, qkv_b, out_w, out_b, _trace=False):
    if _trace:
        _ensure_ntff_hook()
    x = np.asarray(x, dtype=np.float32)
    qkv_w = np.asarray(qkv_w, dtype=np.float32)
    qkv_b = np.asarray(qkv_b, dtype=np.float32)
    out_w = np.asarray(out_w, dtype=np.float32)
    out_b = np.asarray(out_b, dtype=np.float32)

    has_qkv_bias = bool(np.any(qkv_b[:2 * D]))
    nc = _get_nc(has_qkv_bias)
    in_maps = make_in_maps(x, qkv_w, qkv_b, out_w)
    res = run_bass_kernel_spmd(nc, in_maps, core_ids=list(range(NCORES)),
                               trace=_trace)
    y = np.zeros((B, S, D), dtype=np.float32)
    for c in range(NCORES):
        y[c // (NCORES // B)] += np.asarray(res.results[c]["y"],
                                            dtype=np.float32)
    y += out_b
    if _trace:
        kernel.last_results = res
    return y
